# revision 11
# baseline (speedup 1.0000x reference)
"""Trainium2 Bass kernel for nn_Attention_18940805775470.

8-sample batch of a per-sample attention block (EfficientViT-style
cascaded-group-attention cell):
  qkv 1x1 conv + BN -> 8-head attention (kd=16, hd=32, n=1024 tokens)
  -> + depthwise 3x3 BN branch on v -> 1x1 proj + BN.

Distribution: data-parallel, one sample per NeuronCore (B=8 == 8 cores).
All BN folds are done host-side; device does bf16 matmuls with fp32 PSUM
accumulation.

Per-core device algorithm (sample = x [256, 1024]):
  qpack/kpack: qkv weights row-permuted so that head (4g+c)'s q (scaled by
      1/sqrt(kd), bias folded in) lands on SBUF partitions 32c..32c+16 of
      tile g -- the layout the 4-way row-tiled (K=16) scores matmuls need.
  scores S2[j,i] = k^T q per head, 4 heads concurrently via
      tile_position=(32c, 0); softmax without max-subtraction (|S| < 9).
  exp on ScalarE straight out of PSUM ([128, 2048] per instruction).
  ON[d,i] = v0 @ E2 and sums s[i] = 1^T E2 via 4-way column tiling
      (tile_position=(0, 32c)) -- the ones-lhsT is [128, 32] so the sums
      arrive pre-broadcast over each head's 32 partitions.
  x_attn = ON * (1/s) + pe, with pe the depthwise 3x3 conv computed as 9
      accumulating diagonal matmuls over a zero-padded [128, 34*34] copy
      of v.  v's BN bias enters the attention path exactly as +bv (softmax
      rows sum to 1), so it is folded into the proj bias instead.
  out = proj @ x_attn + bias_final.
"""

import sys

sys.path.insert(0, "/opt/trn_rl_repo")

import numpy as np
import ml_dtypes

BF16 = ml_dtypes.bfloat16

DIM = 256
NH = 8
HD = 32
KD = 16
SCALE = KD ** -0.5
EPS = 1e-3
B = 8
N = 1024  # 32*32 tokens
NCORES = 8
NGRP = 2  # head groups of 4

_CACHE = {}


def _build_host_weights(qkv_w, qkv_g, qkv_b, qkv_m, qkv_v,
                        pe_w, pe_g, pe_b, pe_m, pe_v,
                        proj_w, proj_g, proj_b, proj_m, proj_v):
    """Fold BN into weights and build the device-layout arrays."""
    inv_qkv = qkv_g / np.sqrt(qkv_v + EPS)
    Wq_full = qkv_w * inv_qkv[:, None]          # [512, 256]
    bq_full = qkv_b - qkv_m * inv_qkv           # [512]

    inv_pe = pe_g / np.sqrt(pe_v + EPS)
    bpe = pe_b - pe_m * inv_pe                  # [256]
    wpe = pe_w[:, 0] * inv_pe[:, None, None]    # [256, 3, 3]

    inv_p = proj_g / np.sqrt(proj_v + EPS)
    Pw = proj_w * inv_p[:, None]                # [256, 256]
    bp = proj_b - proj_m * inv_p                # [256]

    # q/k packed weight tiles: [128, NGRP*2*128]; block (g, kc) holds
    # lhsT [cc, m] with m = 32c + t.
    wq = np.zeros((128, NGRP * 2 * 128), np.float32)
    wk = np.zeros((128, NGRP * 2 * 128), np.float32)
    bqp = np.zeros((128, NGRP), np.float32)
    for g in range(NGRP):
        for c in range(4):
            h = 4 * g + c
            for kc in range(2):
                col0 = (g * 2 + kc) * 128
                # q rows (scaled); t in [0,16)
                wq[:, col0 + 32 * c: col0 + 32 * c + KD] = \
                    SCALE * Wq_full[h * 64: h * 64 + KD,
                                    kc * 128:(kc + 1) * 128].T
                # k rows, packed at the strip base like q (device reads
                # rows 32c..32c+16 of the separate kp tile)
                wk[:, col0 + 32 * c: col0 + 32 * c + KD] = \
                    Wq_full[h * 64 + KD: h * 64 + 2 * KD,
                            kc * 128:(kc + 1) * 128].T
            bqp[32 * c: 32 * c + KD, g] = \
                SCALE * bq_full[h * 64: h * 64 + KD]

    # v weights, channel-major (c = h*32 + d), transposed for lhsT/rhs use.
    vrows = np.array([(o // HD) * 64 + 2 * KD + (o % HD) for o in range(DIM)])
    Wv = Wq_full[vrows]                         # [256, 256]
    bv = bq_full[vrows]                         # [256]
    wv = np.zeros((128, 2 * 256), np.float32)   # [cc, kc*256 + o]
    for kc in range(2):
        wv[:, kc * 256:(kc + 1) * 256] = Wv[:, kc * 128:(kc + 1) * 128].T

    # depthwise conv diag tiles: [128, 2*9*128]
    dg = np.zeros((128, 2 * 9 * 128), np.float32)
    idx = np.arange(128)
    for ct in range(2):
        for tap in range(9):
            dy, dx = tap // 3, tap % 3
            blk = (ct * 9 + tap) * 128
            dg[idx, blk + idx] = wpe[ct * 128 + idx, dy, dx]

    # proj lhsT tiles: [128, (kc*2 + oc)*128 + o]
    pp = np.zeros((128, 4 * 128), np.float32)
    for kc in range(2):
        for oc in range(2):
            pp[:, (kc * 2 + oc) * 128:(kc * 2 + oc + 1) * 128] = \
                Pw[oc * 128:(oc + 1) * 128, kc * 128:(kc + 1) * 128].T

    bias_final = bp + Pw @ (bpe + bv)           # [256]

    bias_mat = np.zeros((128, 8), np.float32)
    bias_mat[:, 0:2] = bqp
    bias_mat[:, 2] = bv[:128]
    bias_mat[:, 3] = bv[128:]
    bias_mat[:, 4] = bias_final[:128]
    bias_mat[:, 5] = bias_final[128:]

    return {
        "wq": wq.astype(BF16),
        "wk": wk.astype(BF16),
        "wv": wv.astype(BF16),
        "dg": dg.astype(BF16),
        "pp": pp.astype(BF16),
        "bias": bias_mat,
    }


def _build_module():
    import concourse.bass as bass
    import concourse.mybir as mybir
    import concourse.tile as tile
    from concourse import bacc

    fp32 = mybir.dt.float32
    bf16 = mybir.dt.bfloat16
    fp16 = mybir.dt.float16
    AF = mybir.ActivationFunctionType
    ALU = mybir.AluOpType

    nc = bacc.Bacc("TRN2", target_bir_lowering=False, debug=False,
                   num_devices=NCORES)

    x_d = nc.dram_tensor("x_bf", [DIM, N], bf16, kind="ExternalInput")
    wq_d = nc.dram_tensor("wq", [128, NGRP * 2 * 128], bf16, kind="ExternalInput")
    wk_d = nc.dram_tensor("wk", [128, NGRP * 2 * 128], bf16, kind="ExternalInput")
    wv_d = nc.dram_tensor("wv", [128, 2 * 256], bf16, kind="ExternalInput")
    dg_d = nc.dram_tensor("dg", [128, 2 * 9 * 128], bf16, kind="ExternalInput")
    pp_d = nc.dram_tensor("pp", [128, 4 * 128], bf16, kind="ExternalInput")
    bias_d = nc.dram_tensor("bias", [128, 8], fp32, kind="ExternalInput")
    y_d = nc.dram_tensor("y", [DIM, N], fp32, kind="ExternalOutput")

    with tile.TileContext(nc) as tc:
        from contextlib import ExitStack
        with ExitStack() as ctx:
            const = ctx.enter_context(tc.tile_pool(name="const", bufs=1))
            work = ctx.enter_context(tc.tile_pool(name="work", bufs=1))

            # ---- load inputs/weights ----
            xb = []
            for kc in range(2):
                t = const.tile([128, N], bf16, tag=f"xb{kc}", name=f"xb{kc}")
                nc.gpsimd.dma_start(t[:], x_d[kc * 128:(kc + 1) * 128, :])
                xb.append(t)
            wq_sb = const.tile([128, NGRP * 2 * 128], bf16, tag="wq")
            nc.gpsimd.dma_start(wq_sb[:], wq_d[:])
            wk_sb = const.tile([128, NGRP * 2 * 128], bf16, tag="wk")
            nc.gpsimd.dma_start(wk_sb[:], wk_d[:])
            wv_sb = const.tile([128, 2 * 256], bf16, tag="wv")
            nc.gpsimd.dma_start(wv_sb[:], wv_d[:])
            dg_sb = const.tile([128, 2 * 9 * 128], bf16, tag="dg")
            nc.gpsimd.dma_start(dg_sb[:], dg_d[:])
            pp_sb = const.tile([128, 4 * 128], bf16, tag="pp")
            nc.gpsimd.dma_start(pp_sb[:], pp_d[:])
            bias_sb = const.tile([128, 8], fp32, tag="bias")
            nc.gpsimd.dma_start(bias_sb[:], bias_d[:])

            ones_sb = const.tile([128, 32], bf16, tag="ones")
            nc.vector.memset(ones_sb[:], 1.0)

            # ---- persistent intermediate tiles ----
            qp_sb = [const.tile([128, N], bf16, tag=f"qp{g}", name=f"qp{g}") for g in range(NGRP)]
            kp_sb = [const.tile([128, N], bf16, tag=f"kp{g}", name=f"kp{g}") for g in range(NGRP)]
            vt_sb = [const.tile([128, DIM], bf16, tag=f"vt{pc}", name=f"vt{pc}") for pc in range(8)]
            vpad = [const.tile([128, 34 * 34], bf16, tag=f"vpad{ct}", name=f"vpad{ct}") for ct in range(2)]
            pe_sb = [const.tile([128, N], bf16, tag=f"pe{ct}", name=f"pe{ct}") for ct in range(2)]
            xattn = [const.tile([128, N], bf16, tag=f"xat{ct}", name=f"xat{ct}") for ct in range(2)]
            out_sb = [const.tile([128, N], fp32, tag=f"out{oc}", name=f"out{oc}") for oc in range(2)]

            for ct in range(2):
                nc.vector.memset(vpad[ct][:], 0.0)

            # ================= phase 1: qkv projections =================
            # Emission order is PE program order; keep the prologue minimal
            # so the scores/exp pipeline starts early: qk(g0) -> vT -> v ->
            # qk(g1).  The depthwise conv is deferred into the attention
            # loop's PE slack.
            with tc.tile_pool(name="ps1", bufs=2, space="PSUM") as ps1:
                def qkpack(g):
                    for nc2 in range(2):
                        sl = slice(nc2 * 512, (nc2 + 1) * 512)
                        pq = ps1.tile([128, 512], fp32, tag="ps1", name="ps1")
                        for kc in range(2):
                            col = (g * 2 + kc) * 128
                            nc.tensor.matmul(
                                pq[:], wq_sb[:, col:col + 128], xb[kc][:, sl],
                                start=(kc == 0), stop=(kc == 1))
                        nc.vector.tensor_scalar_add(
                            qp_sb[g][:, sl], pq[:], bias_sb[:, g:g + 1])
                        pk = ps1.tile([128, 512], fp32, tag="ps1", name="ps1")
                        for kc in range(2):
                            col = (g * 2 + kc) * 128
                            nc.tensor.matmul(
                                pk[:], wk_sb[:, col:col + 128], xb[kc][:, sl],
                                start=(kc == 0), stop=(kc == 1))
                        nc.vector.tensor_copy(kp_sb[g][:, sl], pk[:])

                qkpack(0)
                # vT (un-biased), p-major -- needed by the first V matmuls
                for pc in range(8):
                    pvt = ps1.tile([128, 256], fp32, tag="ps1", name="ps1")
                    for kc in range(2):
                        nc.tensor.matmul(
                            pvt[:], xb[kc][:, pc * 128:(pc + 1) * 128],
                            wv_sb[:, kc * 256:(kc + 1) * 256],
                            start=(kc == 0), stop=(kc == 1))
                    nc.vector.tensor_copy(vt_sb[pc][:], pvt[:])
                # v (spatial, biased, into padded layout) -- feeds pe conv
                for ct in range(2):
                    vp3 = vpad[ct][:].rearrange("p (a b) -> p a b", a=34)
                    for nc2 in range(2):
                        y0 = nc2 * 16
                        pv = ps1.tile([128, 512], fp32, tag="ps1", name="ps1")
                        for kc in range(2):
                            nc.tensor.matmul(
                                pv[:],
                                wv_sb[:, kc * 256 + ct * 128: kc * 256 + ct * 128 + 128],
                                xb[kc][:, nc2 * 512:(nc2 + 1) * 512],
                                start=(kc == 0), stop=(kc == 1))
                        nc.vector.tensor_scalar_add(
                            vp3[:, 1 + y0:1 + y0 + 16, 1:33],
                            pv[:].rearrange("p (a b) -> p a b", b=32),
                            bias_sb[:, 2 + ct:3 + ct])
                qkpack(1)

            # ============ phase 2+3: attention, proj per i-chunk ============
            # Scores are computed per head-PAIR into [128, 1024] (2-bank)
            # tiles, double-buffered, so the PE produces scores(jc+1) while
            # ACT exps jc and the exp stream never stalls.  V(jc-1)/
            # sums(jc-1) fill the remaining PE slack, and the deferred
            # depthwise-conv jobs are drip-fed into the loop.
            with tc.tile_pool(name="scps", bufs=2, space="PSUM") as scps, \
                 tc.tile_pool(name="onps", bufs=1, space="PSUM") as onps, \
                 tc.tile_pool(name="sps", bufs=1, space="PSUM") as sps, \
                 tc.tile_pool(name="peps", bufs=1, space="PSUM") as peps, \
                 tc.tile_pool(name="e2", bufs=6) as e2p, \
                 tc.tile_pool(name="nrm", bufs=2) as nrm:

                def pe_job(ct, nc2):
                    vp3 = vpad[ct][:].rearrange("p (a b) -> p a b", a=34)
                    y0 = nc2 * 16
                    pp_ps = peps.tile([128, 512], fp32, tag="peps", name="peps")
                    for tap in range(9):
                        dy, dx = tap // 3, tap % 3
                        blk = (ct * 9 + tap) * 128
                        nc.tensor.matmul(
                            pp_ps[:], dg_sb[:, blk:blk + 128],
                            vp3[:, y0 + dy:y0 + dy + 16, dx:dx + 32],
                            start=(tap == 0), stop=(tap == 8))
                    nc.vector.tensor_copy(
                        pe_sb[ct][:, nc2 * 512:(nc2 + 1) * 512], pp_ps[:])

                pe_jobs = [(ct, nc2) for ct in range(2) for nc2 in range(2)]

                for ic in range(2):
                    isl = slice(ic * 512, (ic + 1) * 512)
                    for g in range(NGRP):
                        e2 = []
                        on_ps = onps.tile([128, 512], fp32, tag="on", name="on")
                        s_ps = sps.tile([128, 512], fp32, tag="s", name="s")

                        def vsums(jc):
                            for c in range(4):
                                h = 4 * g + c
                                nc.tensor.matmul(
                                    on_ps[32 * c:32 * c + 32, :],
                                    vt_sb[jc][:, h * 32:(h + 1) * 32],
                                    e2[jc][c // 2][:, (c % 2) * 512:(c % 2) * 512 + 512],
                                    start=(jc == 0), stop=(jc == 7),
                                    tile_position=(0, 32 * c),
                                    skip_group_check=True)
                            for c in range(4):
                                nc.tensor.matmul(
                                    s_ps[32 * c:32 * c + 32, :],
                                    ones_sb[:],
                                    e2[jc][c // 2][:, (c % 2) * 512:(c % 2) * 512 + 512],
                                    start=(jc == 0), stop=(jc == 7),
                                    tile_position=(0, 32 * c),
                                    skip_group_check=True)

                        for jc in range(8):
                            pair = []
                            for half in range(2):
                                sc = scps.tile([128, 1024], fp32, tag="sc",
                                               name="sc")
                                for cc in range(2):
                                    c = half * 2 + cc
                                    nc.tensor.matmul(
                                        sc[:, cc * 512:(cc + 1) * 512],
                                        kp_sb[g][32 * c:32 * c + KD,
                                                 jc * 128:(jc + 1) * 128],
                                        qp_sb[g][32 * c:32 * c + KD, isl],
                                        start=True, stop=True,
                                        tile_position=(32 * c, 0))
                                e = e2p.tile([128, 1024], bf16, tag="e2",
                                             name="e2")
                                nc.scalar.activation(e[:], sc[:], AF.Exp)
                                pair.append(e)
                            e2.append(pair)
                            if jc >= 2 and jc % 2 == 0:
                                vsums(jc - 2)
                                vsums(jc - 1)
                            if jc in (2, 5) and pe_jobs:
                                pe_job(*pe_jobs.pop(0))
                        vsums(6)
                        vsums(7)
                        rbc = nrm.tile([128, 512], fp32, tag="rbc", name="rbc")
                        nc.vector.reciprocal_approx_fast(rbc[:], s_ps[:])
                        tmp = nrm.tile([128, 512], bf16, tag="tmp", name="tmp")
                        nc.vector.tensor_mul(tmp[:], on_ps[:], rbc[:])
                        nc.vector.tensor_add(
                            xattn[g][:, isl], tmp[:], pe_sb[g][:, isl])

                    # proj for this i-chunk; pj shares the "on" pool slot
                    for oc in range(2):
                        pj = onps.tile([128, 512], fp32, tag="on", name="pj")
                        for kc in range(2):
                            col = (kc * 2 + oc) * 128
                            nc.tensor.matmul(
                                pj[:], pp_sb[:, col:col + 128],
                                xattn[kc][:, isl],
                                start=(kc == 0), stop=(kc == 1))
                        nc.vector.tensor_scalar_add(
                            out_sb[oc][:, isl], pj[:], bias_sb[:, 4 + oc:5 + oc])
                        nc.gpsimd.dma_start(
                            y_d[oc * 128:(oc + 1) * 128, isl],
                            out_sb[oc][:, isl])

    nc.compile()
    return nc


def _get_module():
    if "nc" not in _CACHE:
        _CACHE["nc"] = _build_module()
    return _CACHE["nc"]


def kernel(x, qkv_w, qkv_g, qkv_b, qkv_m, qkv_v,
           pe_w, pe_g, pe_b, pe_m, pe_v,
           proj_w, proj_g, proj_b, proj_m, proj_v,
           _trace=False, _trace_kwargs=None):
    from concourse.bass_utils import run_bass_kernel_spmd

    w = _build_host_weights(
        np.asarray(qkv_w, np.float32), np.asarray(qkv_g, np.float32),
        np.asarray(qkv_b, np.float32), np.asarray(qkv_m, np.float32),
        np.asarray(qkv_v, np.float32),
        np.asarray(pe_w, np.float32), np.asarray(pe_g, np.float32),
        np.asarray(pe_b, np.float32), np.asarray(pe_m, np.float32),
        np.asarray(pe_v, np.float32),
        np.asarray(proj_w, np.float32), np.asarray(proj_g, np.float32),
        np.asarray(proj_b, np.float32), np.asarray(proj_m, np.float32),
        np.asarray(proj_v, np.float32))

    x = np.asarray(x, np.float32)
    in_maps = []
    for b in range(B):
        m = dict(w)
        m["x_bf"] = x[b].reshape(DIM, N).astype(BF16)
        in_maps.append(m)

    nc = _get_module()
    res = run_bass_kernel_spmd(nc, in_maps, core_ids=list(range(NCORES)),
                               trace=_trace, **(_trace_kwargs or {}))
    out = np.stack([res.results[b]["y"].reshape(DIM, 32, 32)
                    for b in range(B)])
    if _trace:
        return out.astype(np.float32), res
    return out.astype(np.float32)
